# revision 42
# baseline (speedup 1.0000x reference)
"""Trainium2 Bass kernel for NT-Xent contrastive loss (BATCH=4096, DIM=512, TEMP=0.5).

Strategy (data-parallel over rows of the 2B x 2B similarity matrix):
  - Host: E = concat(emb_i, emb_j) [8192, 512] f32. Each core gets
    ET = E.T bf16 (replicated), its own 1024-row block as fp8e4 in
    DoubleRow k-pair layout, row-major copies of own + partner blocks
    (bf16), and two tiny constant matrices for PE column sums / selects.
  - Device (per core, SPMD, no collectives):
      * row sumsq of all 8192 rows WITHOUT a row-major copy: DVE/GpSimd
        squares of the transposed tiles + PE one-hot matmuls that
        column-sum them (weights scaled 1/256 so the batched ACT
        exp(-0.5*ln(x)) rsqrt directly yields 16/||e||).
      * r-row broadcast across partitions via one-hot PE matmul to PSUM,
        fast tensor_copy to SBUF bf16 (frees the PSUM ring), then DVE
        normalize: etn = et * (16 r) in fp8e4.
      * main similarity matmul in fp8 DoubleRow (2 k-chunks of 128 per
        instruction, 2x PE throughput), fp32 PSUM accum.
      * ACT: exp(S' * r_row/(16*TEMP)) with fused row-sum accumulation.
      * positives via DVE row-dots of own x partner row-major blocks.
      * per-core partial: sum_rows(log(den - e^{1/TEMP}) - pos/TEMP).
  - Host: loss = sum(partials) / (2B).

The column groups are processed as 5 batches [(0,),(1,),(2,3),(4,5),(6,7)]
-- two narrow batches up front shorten the startup chain.  Chains for a
batch (squares -> colsum -> rsqrt -> broadcast -> normalize) are emitted
into earlier batches' instruction streams; per-engine FIFO order is the
scheduler.
"""

import math

import ml_dtypes
import numpy as np

BATCH = 4096
DIM = 512
TEMP = 0.5
B2 = 2 * BATCH              # 8192 rows/cols of the similarity matrix
NCORES = 8
RPC = B2 // NCORES          # 1024 rows per core
KT = DIM // 128             # 4 contraction chunks
CG = 8                      # column groups
CGW = B2 // CG              # 1024 columns per group
T8 = RPC // 128             # 8 row-tiles per core
BATCHES = [(0,), (1,), (2, 3), (4, 5), (6, 7)]
NB = len(BATCHES)
EXP_DIAG = math.exp(1.0 / TEMP)
S16 = 16.0                  # fp8 range scale folded into r

_CACHE = {}


def _build():
    import concourse.bacc as bacc
    import concourse.mybir as mybir
    import concourse.tile as tile

    f32 = mybir.dt.float32
    bf16 = mybir.dt.bfloat16
    f8 = mybir.dt.float8e4
    AF = mybir.ActivationFunctionType
    ALU = mybir.AluOpType
    X = mybir.AxisListType.X
    DR = mybir.MatmulPerfMode.DoubleRow

    import bass_rust as _bass_rust
    from concourse.hw_specs import get_activation_tables

    class _Bacc(bacc.Bacc):
        """Bacc that pins Exp+Ln to the combined natural_log_exp_and_others
        activation-table set, so the kernel never swaps ACT tables."""

        def insert_act_table_loads(self):
            has_activation = any(
                isinstance(i, mybir.InstActivation)
                for b in self.main_func.blocks
                for i in b.instructions)
            if not has_activation:
                return
            drop = {mybir.ActivationFunctionType.Exp,
                    mybir.ActivationFunctionType.Ln}
            tables = []
            for name, funcs in get_activation_tables(self.m.arch).items():
                if name != "natural_log_exp_and_others":
                    funcs = funcs - drop
                tables.append((name, funcs))
            _bass_rust.insert_act_table_loads(self, tables)

    nc = _Bacc("TRN2", target_bir_lowering=False, debug=False,
               num_devices=NCORES)

    et_d = nc.dram_tensor("et", [DIM, B2], bf16, kind="ExternalInput").ap()
    etbp_d = nc.dram_tensor("etbp", [128, KT * RPC], f8,
                            kind="ExternalInput").ap()
    ermb_d = nc.dram_tensor("ermb", [128, T8 * DIM], bf16,
                            kind="ExternalInput").ap()
    ermp_d = nc.dram_tensor("ermp", [128, T8 * DIM], bf16,
                            kind="ExternalInput").ap()
    csel_d = nc.dram_tensor("csel", [128, 4], bf16, kind="ExternalInput").ap()
    bsel_d = nc.dram_tensor("bsel", [2, 256], bf16, kind="ExternalInput").ap()
    out_d = nc.dram_tensor("out", [1, 1], f32, kind="ExternalOutput").ap()

    with tile.TileContext(nc) as tc:
        with (
            tc.tile_pool(name="persist", bufs=1) as P,
            tc.tile_pool(name="scratch", bufs=2) as S,
            tc.tile_pool(name="psum", bufs=2, space="PSUM") as PS,
        ):
            ssb = P.tile([128, T8], f32, name="ssb")
            ssp = P.tile([128, T8], f32, name="ssp")
            rawpos = P.tile([128, T8], f32, name="rawpos")
            rsums = P.tile([128, T8 * NB], f32, name="rsums")
            sc8 = P.tile([128, T8], f32, name="sc8")
            pos8 = P.tile([128, T8], f32, name="pos8")
            rb8 = P.tile([128, T8], f32, name="rb8")
            rp8 = P.tile([128, T8], f32, name="rp8")
            ones = P.tile([128, 1], f32, name="ones")
            csel = P.tile([128, 4], bf16, name="csel")
            bsel = P.tile([2, 256], bf16, name="bsel")
            rsqg = [P.tile([1, CGW], bf16, name="rsqg_0"),
                    P.tile([1, CGW], bf16, name="rsqg_1")]
            rsqb = [P.tile([2, CGW], bf16, name=f"rsqb_{i}")
                    for i in range(3)]
            etbp = P.tile([128, KT * RPC], f8, name="etbp")
            ermb = P.tile([128, T8 * DIM], bf16, name="ermb")
            ermp = P.tile([128, T8 * DIM], bf16, name="ermp")
            etn2 = [[P.tile([128, 2 * CGW], f8, name=f"etn_{kp}_{c}")
                     for c in range(CG)] for kp in range(2)]
            et2 = [[None] * CG for _ in range(KT)]

            nc.vector.memset(ones[:], 1.0)
            etbp4 = etbp[:].rearrange("p (a m) -> p a m", a=KT)

            def load_et(c):
                for k in range(KT):
                    et2[k][c] = S.tile([128, CGW], bf16, name=f"et_{k}_{c}",
                                       tag="etraw", bufs=32)
                    nc.sync.dma_start(
                        et2[k][c][:],
                        et_d[k * 128:(k + 1) * 128, c * CGW:(c + 1) * CGW])

            sq = [[None] * CG for _ in range(KT)]

            def squares(c, split=False):
                for k in range(KT):
                    sq[k][c] = S.tile([128, CGW], bf16, name=f"sq_{k}_{c}",
                                      tag="sq", bufs=16)
                    nc.vector.tensor_tensor(sq[k][c][:], et2[k][c][:],
                                            et2[k][c][:], ALU.mult)

            def colsum_solo(g):
                """PE one-hot column sums (x 1/256) for one group -> rsqg."""
                ssq = PS.tile([1, CGW], f32, tag="mm", name="ssq1")
                for k in range(KT):
                    for h in range(2):
                        nc.tensor.matmul(
                            ssq[:, h * 512:(h + 1) * 512],
                            csel[:, 0:1],
                            sq[k][g][:, h * 512:(h + 1) * 512],
                            start=(k == 0), stop=(k == KT - 1))
                lnb = S.tile([2, CGW], f32, tag="lnb", name="lnb")
                nc.scalar.activation(lnb[0:1, :], ssq[:], AF.Ln)
                nc.scalar.activation(rsqg[g][:], lnb[0:1, :],
                                     AF.Exp, scale=-0.5)

            def colsum_pair(pi):
                """Column sums for group pair (2+2*pi, 3+2*pi) -> rsqb[pi]."""
                ssq = PS.tile([2, CGW], f32, tag="mm", name="ssq2")
                for gi in range(2):
                    c = 2 + 2 * pi + gi
                    for k in range(KT):
                        for h in range(2):
                            nc.tensor.matmul(
                                ssq[:, h * 512:(h + 1) * 512],
                                csel[:, 2 * gi:2 * gi + 2],
                                sq[k][c][:, h * 512:(h + 1) * 512],
                                start=(gi == 0 and k == 0),
                                stop=(gi == 1 and k == KT - 1))
                lnb = S.tile([2, CGW], f32, tag="lnb", name="lnb")
                nc.scalar.activation(lnb[:], ssq[:], AF.Ln)
                nc.scalar.activation(rsqb[pi][:], lnb[:], AF.Exp, scale=-0.5)

            def bcast_norm(c):
                """Replicate 16/||e|| row across partitions with a one-hot
                matmul, tensor_copy it off PSUM (frees the ring fast), then
                normalize the transposed tiles into fp8 (DVE)."""
                pb = PS.tile([128, CGW], f32, tag="mm", name="pb")
                if c < 2:
                    lhsT, rsq = bsel[0:1, 0:128], rsqg[c][:]
                else:
                    gi = c % 2
                    lhsT, rsq = (bsel[:, gi * 128:(gi + 1) * 128],
                                 rsqb[(c - 2) // 2][:])
                for h in range(2):
                    nc.tensor.matmul(pb[:, h * 512:(h + 1) * 512], lhsT,
                                     rsq[:, h * 512:(h + 1) * 512],
                                     start=True, stop=True)
                rbc = S.tile([128, CGW], bf16, tag="rbc", bufs=4, name="rbc")
                nc.vector.tensor_copy(rbc[:], pb[:])
                for k in range(KT):
                    nc.vector.tensor_tensor(
                        etn2[k // 2][c][:, (k % 2) * CGW:(k % 2 + 1) * CGW],
                        et2[k][c][:], rbc[:], ALU.mult)

            def sumsq_rm(src, tt, dst, dcol, src2=None):
                sco = S.tile([128, DIM], bf16, tag="stt", name="sco")
                s2 = src2 if src2 is not None else src
                nc.vector.scalar_tensor_tensor(
                    sco[:], src[:, tt * DIM:(tt + 1) * DIM], 1.0,
                    s2[:, tt * DIM:(tt + 1) * DIM], ALU.mult, ALU.mult,
                    accum_out=dst[:, dcol:dcol + 1])

            def rsqrt8(dst, src):
                ln = S.tile([128, T8], f32, tag="ln8", name="ln8")
                nc.scalar.activation(ln[:], src[:], AF.Ln)
                nc.scalar.activation(dst[:], ln[:], AF.Exp, scale=-0.5)

            def main_tile(bi, cgs, t):
                wid = len(cgs) * CGW
                ps = PS.tile([128, wid], f32, tag="mm", name="psmm")
                for kp in range(2):
                    lhsT = etbp4[:, 2 * kp:2 * kp + 2, t * 128:(t + 1) * 128]
                    for ci, c in enumerate(cgs):
                        pair = etn2[kp][c][:].rearrange(
                            "p (two f) -> p two f", two=2)
                        for n in range(2):
                            nc.tensor.matmul(
                                ps[:, ci * CGW + n * 512:
                                   ci * CGW + (n + 1) * 512],
                                lhsT, pair[:, :, n * 512:(n + 1) * 512],
                                start=(kp == 0), stop=(kp == 1),
                                perf_mode=DR)
                sce = S.tile([128, wid], bf16, tag="expout", name="sce")
                nc.scalar.activation(sce[:], ps[:], AF.Exp,
                                     scale=sc8[:, t:t + 1],
                                     accum_out=rsums[:, t * NB + bi:
                                                     t * NB + bi + 1])

            # ---- paced emission ----
            load_et(0)
            load_et(1)
            nc.sync.dma_start(csel[:], csel_d[:])
            nc.sync.dma_start(bsel[:], bsel_d[:])
            nc.sync.dma_start(ermb[:], ermb_d[:])
            nc.sync.dma_start(etbp[:], etbp_d[:])
            load_et(2)
            load_et(3)
            squares(0)
            colsum_solo(0)
            bcast_norm(0)
            squares(1)
            for t in range(T8):                      # own norms -> ACT scale
                sumsq_rm(ermb, t, ssb, t)
            rsqrt8(rb8, ssb)
            nc.vector.tensor_scalar_mul(sc8[:], rb8[:], 1.0 / (S16 * TEMP))
            load_et(4)
            load_et(5)
            nc.sync.dma_start(ermp[:], ermp_d[:])

            for bi, cgs in enumerate(BATCHES):
                for t in range(T8):
                    main_tile(bi, cgs, t)
                    if bi == 0:
                        if t == 1:
                            colsum_solo(1)
                        if t == 2:
                            bcast_norm(1)
                        if t == 3:
                            squares(2, split=True)
                            squares(3, split=True)
                        if t == 7:
                            colsum_pair(0)           # groups 2,3
                    if bi == 1:
                        if t == 1:
                            bcast_norm(2)
                        if t == 3:
                            bcast_norm(3)
                        if t == 5:
                            squares(4, split=True)
                            squares(5, split=True)
                    if bi == 2:
                        if t == 1:
                            load_et(6)
                            load_et(7)
                        if t == 3:
                            colsum_pair(1)           # groups 4,5
                        if t == 5:
                            bcast_norm(4)
                        if t == 6:
                            squares(6, split=True)
                            squares(7, split=True)
                        if t == 7:
                            bcast_norm(5)
                    if bi == 3:
                        if t == 1:
                            colsum_pair(2)           # groups 6,7
                        if t == 3:
                            bcast_norm(6)
                        if t == 5:
                            bcast_norm(7)
                    if bi == 4:                      # partner path, off-crit
                        if t == 0:
                            for tt in range(T8):
                                sumsq_rm(ermp, tt, ssp, tt)
                        if t == 2:
                            for tt in range(T8):
                                sumsq_rm(ermb, tt, rawpos, tt, src2=ermp)
                        if t == 5:
                            rsqrt8(rp8, ssp)
                            pt0 = P.tile([128, T8], f32, name="pt0")
                            nc.vector.tensor_mul(pt0[:], rawpos[:], rb8[:])
                            pt1 = P.tile([128, T8], f32, name="pt1")
                            nc.vector.tensor_mul(pt1[:], pt0[:], rp8[:])
                            nc.vector.tensor_scalar_mul(pos8[:], pt1[:],
                                                        1.0 / TEMP)

            # ---- finalize: den = rowsum - e^{1/T}; sum(log(den) - pos) ----
            den8 = P.tile([128, T8], f32, name="den8")
            nc.vector.tensor_reduce(
                den8[:], rsums[:].rearrange("p (t c) -> p t c", c=NB),
                X, ALU.add)
            den8b = P.tile([128, T8], f32, name="den8b")
            nc.vector.tensor_scalar_add(den8b[:], den8[:], -EXP_DIAG)
            logd = S.tile([128, T8], f32, tag="logd", name="logd")
            tlog = P.tile([128, 1], f32, name="tlog")
            nc.scalar.activation(logd[:], den8b[:], AF.Ln, accum_out=tlog[:])
            tpos = P.tile([128, 1], f32, name="tpos")
            nc.vector.tensor_reduce(tpos[:], pos8[:], X, ALU.add)
            lv = P.tile([128, 1], f32, name="lv")
            nc.vector.tensor_sub(lv[:], tlog[:], tpos[:])
            psf = PS.tile([1, 1], f32, tag="mm", name="psf")
            nc.tensor.matmul(psf[:], lv[:], ones[:], start=True, stop=True)
            ob = P.tile([1, 1], f32, name="ob")
            nc.vector.tensor_copy(ob[:], psf[:])
            nc.sync.dma_start(out_d[:], ob[:])

    nc.compile()
    return nc


def _get_nc():
    if "nc" not in _CACHE:
        _CACHE["nc"] = _build()
    return _CACHE["nc"]


def _in_maps(emb_i, emb_j):
    bf = ml_dtypes.bfloat16
    f8 = ml_dtypes.float8_e4m3
    E = np.concatenate([np.asarray(emb_i, dtype=np.float32),
                        np.asarray(emb_j, dtype=np.float32)], axis=0)
    Ebf = E.astype(bf)                              # [8192, 512] row-major
    ET = np.ascontiguousarray(Ebf.T)                # [512, 8192]
    # SBUF-image tiling of the row-major copy: ERMT[p, t*512+d] = Ebf[t*128+p, d]
    ERMT = np.ascontiguousarray(
        Ebf.reshape(B2 // 128, 128, DIM).transpose(1, 0, 2).reshape(128, -1))
    # one-hot column-sum weights, scaled 1/256 (so rsqrt yields 16/||e||)
    CSEL = np.zeros((128, 4), dtype=bf)
    CSEL[:, 0] = 1.0 / 256.0
    CSEL[:, 3] = 1.0 / 256.0
    BSEL = np.zeros((2, 256), dtype=bf)
    BSEL[0, 0:128] = 1.0
    BSEL[1, 128:256] = 1.0
    maps = []
    for k in range(NCORES):
        s = k * RPC
        p = (s + BATCH) % B2
        # own rows, transposed, fp8, DoubleRow k-pair layout:
        # etbp[pp, a*RPC + r] = ET[a*128+pp, s+r]
        ETBP = np.ascontiguousarray(
            ET[:, s:s + RPC].reshape(KT, 128, RPC)
            .transpose(1, 0, 2).reshape(128, KT * RPC)).astype(f8)
        maps.append({
            "et": ET,
            "etbp": ETBP,
            "ermb": np.ascontiguousarray(
                ERMT[:, s // 128 * DIM:(s // 128 + T8) * DIM]),
            "ermp": np.ascontiguousarray(
                ERMT[:, p // 128 * DIM:(p // 128 + T8) * DIM]),
            "csel": CSEL,
            "bsel": BSEL,
        })
    return maps


def _run(emb_i, emb_j, trace=False):
    from concourse.bass_utils import run_bass_kernel_spmd
    nc = _get_nc()
    res = run_bass_kernel_spmd(nc, _in_maps(emb_i, emb_j),
                               list(range(NCORES)), trace=trace)
    total = sum(float(res.results[i]["out"][0, 0]) for i in range(NCORES))
    loss = np.float32(total / B2)
    return loss, res


def kernel(emb_i, emb_j):
    return _run(emb_i, emb_j, trace=False)[0]
